# revision 7
# baseline (speedup 1.0000x reference)
"""2-layer GCN (100k nodes, 3.2M edges) on 8 Trainium2 NeuronCores.

Strategy (node-partitioned, DMA-gather aggregation):
  - Nodes range-partitioned across 8 cores (12500 real + 44 dummy = 12544
    positions per core, plain order, no global permutation).
  - GCN algebra: out = D^-1/2 A_hat D^-1/2 (H W). Features are pre-scaled by
    dinv before the table, aggregated over in-edges, post-scaled by dinv.
    Layer 2 applies W2 after aggregation (linearity), so both layers
    aggregate 16-wide features.
  - Per layer, each core computes its feature shard node-major, AllGathers it
    ([100352, 16] f32 in DRAM), and spreads it to a 256B-row table
    ([100352, 64] f32) because SWDGE dma_gather descriptors are 256B-aligned.
  - Aggregation is two-level to keep DMA descriptor counts near the edge
    count (int16 gather indices only address 32768 rows, so gathers are
    windowed per source core):
      stage 1: per (window w = src core, group of 4 seg-columns) dma_gather
        pulls each edge's source row; nodes are sorted per-window by their
        window-w in-degree so segment widths stay tight; a strided DVE
        reduce produces per-(node, window) partials written to a local DRAM
        partial table pt[w] in window order.
      stage 2: 8 dma_gathers (one per window, uniform 1 slot/node) pull each
        node's 8 partials back in plain node order; DVE adds combine them.
  - Self-loops are ordinary slots in the own-core window. Padding slots point
    at each core's dummy zero row.

All floating-point math (matmuls, rsqrt, aggregation, bias, relu,
log_softmax) runs on device. The host only restructures integers into the
gather index tensors and relayouts inputs.
"""

import numpy as np

import concourse.bass as bass
import concourse.bacc as bacc
import concourse.mybir as mybir
import concourse.tile as tile
from concourse.bass_utils import run_bass_kernel_spmd

N_NODES = 100000
N_FEAT = 512
HIDDEN = 16
N_CLASSES = 64
NCORES = 8
NPC_REAL = 12500          # real nodes per core
NPC = 12544               # padded positions per core (98 * 128)
NBLK = NPC // 128         # 98 seg-columns / blocks of 128 nodes
NT = NCORES * NPC         # global table rows
EL = 64                   # table row width in f32 (256B rows for dma_gather)
SEG = 4                   # seg-columns per stage-1 gather group
SB = 4                    # blocks per tail-compute group
DUMMY = NPC - 1           # each core's last position is a zero row

_cache = {}


# ----------------------------------------------------------------------------
# host-side graph restructuring (integer work only)
# ----------------------------------------------------------------------------

def _preprocess(edge_index):
    src = edge_index[0].astype(np.int64)
    dst = edge_index[1].astype(np.int64)
    c_dst = dst // NPC_REAL
    l_dst = dst - c_dst * NPC_REAL
    w_src = src // NPC_REAL
    l_src = src - w_src * NPC_REAL

    # per-(core, node, window) message counts, incl. self-loop
    key = (c_dst * NPC + l_dst) * 8 + w_src
    m = np.bincount(key, minlength=NCORES * NPC * 8).reshape(NCORES, NPC, 8)
    for c in range(NCORES):
        m[c, :NPC_REAL, c] += 1

    deg = m.sum(axis=2)                      # in-degree incl self (0 for dummy)

    # per-(core, window) node order by window in-degree desc; dummies last
    m2 = m.transpose(0, 2, 1)                # [c, w, node]
    order = np.argsort(-m2, axis=2, kind="stable")
    rank = np.empty_like(order)
    np.put_along_axis(rank, order, np.broadcast_to(np.arange(NPC), order.shape), axis=2)

    # seg widths (max over 128 sorted nodes), unified across cores
    msort = -np.sort(-m2, axis=2)            # [c, w, node] desc
    segD = msort.reshape(NCORES, 8, NBLK, 128).max(axis=3).max(axis=0)  # [w, seg]

    # stage-1 groups of SEG segs with uniform width
    seg0 = list(range(0, NBLK, SEG))
    groups = [(s0, min(SEG, NBLK - s0)) for s0 in seg0]
    NG = len(groups)
    Dwg = np.zeros((8, NG), dtype=np.int64)
    for w in range(8):
        for gi, (s0, ns) in enumerate(groups):
            Dwg[w, gi] = max(1, segD[w, s0 : s0 + ns].max())

    # instruction layout: concat (w, g) index streams; ni = 128*ns*D
    ni = np.zeros((8, NG), dtype=np.int64)
    for w in range(8):
        for gi, (s0, ns) in enumerate(groups):
            ni[w, gi] = 128 * ns * Dwg[w, gi]
    base = np.zeros(8 * NG + 1, dtype=np.int64)
    base[1:] = np.cumsum(ni.ravel())
    I1 = int(base[-1])                        # total stage-1 indices per core

    # per-edge slot assignment
    ekey = (c_dst * 8 + w_src) * NPC + l_dst
    perm = np.argsort(ekey, kind="stable")
    ekey_s = ekey[perm]
    lsrc_s = l_src[perm]
    counts = np.bincount(ekey_s, minlength=NCORES * 8 * NPC)
    starts = np.zeros(NCORES * 8 * NPC + 1, dtype=np.int64)
    starts[1:] = np.cumsum(counts)
    j_within = np.arange(len(ekey_s), dtype=np.int64) - starts[ekey_s]

    cc = ekey_s // (8 * NPC)
    ww = (ekey_s // NPC) % 8
    ll = ekey_s % NPC
    rk = rank[cc, ww, ll]                     # window-sorted position
    seg = rk // 128
    lane = rk % 128
    gi = seg // SEG
    jloc = seg - gi * SEG
    D = Dwg[ww, gi]
    o = (jloc * D + j_within) * 128 + lane    # offset within instruction
    flat = base[ww * NG + gi] + o

    idx1w = np.full((NCORES, 16, I1 // 16), DUMMY, dtype=np.int16)
    idx1w[cc, flat % 16, flat // 16] = lsrc_s.astype(np.int16)

    # self-loop slots
    for c in range(NCORES):
        l = np.arange(NPC_REAL, dtype=np.int64)
        jw = m[c, :NPC_REAL, c] - 1           # last slot in own-core window
        rk = rank[c, c, l]
        seg = rk // 128
        lane = rk % 128
        gi = seg // SEG
        jloc = seg - gi * SEG
        D = Dwg[c, gi]
        flat = base[c * NG + gi] + (jloc * D + jw) * 128 + lane
        idx1w[c, flat % 16, flat // 16] = l.astype(np.int16)

    idx1 = np.tile(idx1w, (1, 8, 1))          # replicate over the 8 q7 cores

    # stage-2: per window, plain-order nodes gather their partial's rank
    NI2 = NPC
    idx2w = np.zeros((NCORES, 16, (8 * NI2) // 16), dtype=np.int16)
    for c in range(NCORES):
        for w in range(8):
            n = np.arange(NPC, dtype=np.int64)
            flat = w * NI2 + (n // 128) * 128 * 0 + (n % 128) + (n // 128) * 128
            # flat within window w stream: slotpos = block j = n//128, lane = n%128
            val = rank[c, w, n].astype(np.int16)
            idx2w[c, flat % 16, flat // 16] = val
    idx2 = np.tile(idx2w, (1, 8, 1))

    # degrees, node-major [128 part, NBLK] -> repeated 16x along free
    deg_pb = np.maximum(deg, 1).reshape(NCORES, NBLK, 128).transpose(0, 2, 1)
    deg_rep = np.repeat(deg_pb, HIDDEN, axis=2).astype(np.int32)

    return {
        "idx1": idx1,
        "idx2": idx2,
        "deg_rep": deg_rep,
        "groups": groups,
        "Dwg": Dwg,
        "ni": ni,
        "base": base,
        "I1": I1,
    }


# ----------------------------------------------------------------------------
# device program
# ----------------------------------------------------------------------------

def _build_program(meta):
    groups = meta["groups"]
    Dwg = meta["Dwg"]
    ni = meta["ni"]
    base = meta["base"]
    I1 = meta["I1"]
    NG = len(groups)
    f32 = mybir.dt.float32
    bf16 = mybir.dt.bfloat16
    DMAX = int(Dwg.max())
    n_phA = (NBLK + SB - 1) // SB

    nc = bacc.Bacc(
        "TRN2", target_bir_lowering=False, debug=False, num_devices=NCORES
    )
    xT = nc.declare_dram_parameter("xT", [N_FEAT, NPC], bf16, isOutput=False)
    idx1_in = nc.declare_dram_parameter(
        "idx1", [128, I1 // 16], mybir.dt.int16, isOutput=False
    )
    idx2_in = nc.declare_dram_parameter(
        "idx2", [128, (8 * NPC) // 16], mybir.dt.int16, isOutput=False
    )
    degrep_in = nc.declare_dram_parameter(
        "degrep", [128, NBLK * HIDDEN], mybir.dt.int32, isOutput=False
    )
    W1r_in = nc.declare_dram_parameter("W1r", [128, 64], bf16, isOutput=False)
    b1r_in = nc.declare_dram_parameter("b1r", [128, NBLK * HIDDEN], f32, isOutput=False)
    W2s_in = nc.declare_dram_parameter("W2s", [HIDDEN, N_CLASSES], f32, isOutput=False)
    b2r_in = nc.declare_dram_parameter("b2r", [128, SB * N_CLASSES], f32, isOutput=False)
    identf_in = nc.declare_dram_parameter("identf", [128, 128], f32, isOutput=False)
    dmask_in = nc.declare_dram_parameter("dmask", [128, 1], f32, isOutput=False)
    out_d = nc.declare_dram_parameter("out", [NBLK, 128, N_CLASSES], f32, isOutput=True)

    q1d = nc.dram_tensor("q1d", [NPC, HIDDEN], f32)
    q2d = nc.dram_tensor("q2d", [NPC, HIDDEN], f32)
    tab16_1 = nc.dram_tensor("tab16_1", [NT, HIDDEN], f32, addr_space="Shared")
    tab16_2 = nc.dram_tensor("tab16_2", [NT, HIDDEN], f32, addr_space="Shared")
    tab64_1 = nc.dram_tensor("tab64_1", [NT, EL], f32)
    tab64_2 = nc.dram_tensor("tab64_2", [NT, EL], f32)
    pt1 = nc.dram_tensor("pt1", [8, NPC, EL], f32)
    pt2 = nc.dram_tensor("pt2", [8, NPC, EL], f32)

    rg = [list(range(NCORES))]

    with tile.TileContext(nc) as tc:
        with (
            tc.tile_pool(name="const", bufs=1) as cp,
            tc.tile_pool(name="xt", bufs=2) as xp,
            tc.tile_pool(name="i1", bufs=3) as ip,
            tc.tile_pool(name="msg", bufs=2) as mp,
            tc.tile_pool(name="m2", bufs=2) as m2p,
            tc.tile_pool(name="work", bufs=3) as wp,
            tc.tile_pool(name="acc", bufs=1) as ap_,
            tc.tile_pool(name="ps", bufs=2, space="PSUM") as pp,
            tc.tile_pool(name="psT", bufs=2, space="PSUM") as ppT,
            tc.tile_pool(name="psO", bufs=2, space="PSUM") as ppO,
        ):
            # ---- constants -------------------------------------------------
            W1r = cp.tile([128, 64], bf16)
            nc.sync.dma_start(out=W1r[:], in_=W1r_in[:])
            b1r = cp.tile([128, NBLK * HIDDEN], f32)
            nc.sync.dma_start(out=b1r[:], in_=b1r_in[:])
            W2s = cp.tile([HIDDEN, N_CLASSES], f32)
            nc.sync.dma_start(out=W2s[:], in_=W2s_in[:])
            b2r = cp.tile([128, SB * N_CLASSES], f32)
            nc.sync.dma_start(out=b2r[:], in_=b2r_in[:])
            identf = cp.tile([128, 128], f32)
            nc.sync.dma_start(out=identf[:], in_=identf_in[:])
            dmask = cp.tile([128, 1], f32)
            nc.sync.dma_start(out=dmask[:], in_=dmask_in[:])
            idx2_sb = cp.tile([128, (8 * NPC) // 16], mybir.dt.int16)
            nc.sync.dma_start(out=idx2_sb[:], in_=idx2_in[:])

            degrep = cp.tile([128, NBLK * HIDDEN], mybir.dt.int32)
            nc.sync.dma_start(out=degrep[:], in_=degrep_in[:])
            dinvr = cp.tile([128, NBLK * HIDDEN], f32)
            nc.vector.tensor_copy(out=dinvr[:], in_=degrep[:])
            nc.vector.reciprocal(out=dinvr[:], in_=dinvr[:])
            nc.scalar.activation(
                out=dinvr[:], in_=dinvr[:], func=mybir.ActivationFunctionType.Sqrt
            )

            # ---- phase A: q1 = (x @ W1) * dinv, node-major -----------------
            for s in range(n_phA):
                b0 = s * SB
                nblk_s = min(SB, NBLK - b0)
                w = nblk_s * 128
                xts = []
                for kc in range(4):
                    xt = xp.tile([128, SB * 128], bf16, tag=f"xt{kc}")
                    nc.sync.dma_start(
                        out=xt[:, :w],
                        in_=xT[kc * 128 : (kc + 1) * 128, b0 * 128 : b0 * 128 + w],
                    )
                    xts.append(xt)
                psF = pp.tile([HIDDEN, SB * 128], f32, tag="psF")
                for kc in range(4):
                    nc.tensor.matmul(
                        out=psF[:, :w],
                        lhsT=W1r[:, kc * HIDDEN : (kc + 1) * HIDDEN],
                        rhs=xts[kc][:, :w],
                        start=(kc == 0),
                        stop=(kc == 3),
                    )
                qf = wp.tile([HIDDEN, SB * 128], f32, tag="qf")
                nc.vector.tensor_copy(out=qf[:, :w], in_=psF[:, :w])
                q_nm = wp.tile([128, SB * HIDDEN], f32, tag="qnm")
                for j in range(nblk_s):
                    b = b0 + j
                    psT = ppT.tile([128, HIDDEN], f32, tag="psT")
                    nc.tensor.transpose(
                        out=psT[:],
                        in_=qf[:, j * 128 : (j + 1) * 128],
                        identity=identf[:HIDDEN, :HIDDEN],
                    )
                    nc.vector.tensor_scalar_mul(
                        out=q_nm[:, j * HIDDEN : (j + 1) * HIDDEN],
                        in0=psT[:],
                        scalar1=dinvr[:, b * HIDDEN : b * HIDDEN + 1],
                    )
                nc.sync.dma_start(
                    out=q1d[b0 * 128 : b0 * 128 + w].rearrange(
                        "(j p) f -> p j f", p=128
                    ),
                    in_=q_nm[:, : nblk_s * HIDDEN].rearrange(
                        "p (j f) -> p j f", f=HIDDEN
                    ),
                )

            # ---- allgather + spread to 256B rows ---------------------------
            nc.gpsimd.collective_compute(
                "AllGather",
                mybir.AluOpType.bypass,
                replica_groups=rg,
                ins=[q1d[:]],
                outs=[tab16_1[:]],
            )
            for w in range(8):
                nc.sync.dma_start(
                    out=tab64_1[w * NPC : (w + 1) * NPC, :HIDDEN],
                    in_=tab16_1[w * NPC : (w + 1) * NPC],
                )

            # ---- two-level aggregation -------------------------------------
            def aggregate(tab64, pt, acc):
                # stage 1: per (window, group) gather + segmented reduce
                for w in range(8):
                    for gi, (s0, ns) in enumerate(groups):
                        D = int(Dwg[w, gi])
                        n_idx = int(ni[w, gi])
                        cb = int(base[w * NG + gi])
                        i1 = ip.tile([128, (128 * SEG * DMAX) // 16],
                                     mybir.dt.int16, tag="i1")
                        nc.sync.dma_start(
                            out=i1[:, : n_idx // 16],
                            in_=idx1_in[:, cb // 16 : (cb + n_idx) // 16],
                        )
                        msg = mp.tile([128, SEG * DMAX * EL], f32, tag="msg")
                        nc.gpsimd.dma_gather(
                            out_ap=msg[:, : ns * D * EL].rearrange(
                                "p (s e) -> p s e", e=EL
                            ),
                            in_ap=tab64[w * NPC : (w + 1) * NPC],
                            idxs_ap=i1[:, : n_idx // 16],
                            num_idxs=n_idx,
                            num_idxs_reg=n_idx,
                            elem_size=EL,
                            single_packet=False,
                        )
                        part = wp.tile([128, SEG * HIDDEN], f32, tag="part")
                        nc.vector.tensor_reduce(
                            out=part[:, : ns * HIDDEN].rearrange(
                                "p (j f) -> p j f", f=HIDDEN
                            ),
                            in_=msg[:, : ns * D * EL].rearrange(
                                "p (j s e) -> p j e s", j=ns, e=EL
                            )[:, :, :HIDDEN, :],
                            axis=mybir.AxisListType.X,
                            op=mybir.AluOpType.add,
                        )
                        nc.sync.dma_start(
                            out=pt[w, s0 * 128 : (s0 + ns) * 128, :HIDDEN].rearrange(
                                "(j p) f -> p j f", p=128
                            ),
                            in_=part[:, : ns * HIDDEN].rearrange(
                                "p (j f) -> p j f", f=HIDDEN
                            ),
                        )
                # stage 2: per window, gather partials in plain order, combine
                for w in range(8):
                    m2 = m2p.tile([128, NBLK * EL], f32, tag="m2")
                    nc.gpsimd.dma_gather(
                        out_ap=m2[:].rearrange("p (s e) -> p s e", e=EL),
                        in_ap=pt[w],
                        idxs_ap=idx2_sb[:, w * (NPC // 16) : (w + 1) * (NPC // 16)],
                        num_idxs=NPC,
                        num_idxs_reg=NPC,
                        elem_size=EL,
                        single_packet=False,
                    )
                    src = m2[:].rearrange("p (j e) -> p j e", e=EL)[:, :, :HIDDEN]
                    if w == 0:
                        nc.vector.tensor_copy(
                            out=acc[:].rearrange("p (j f) -> p j f", f=HIDDEN),
                            in_=src,
                        )
                    else:
                        nc.vector.tensor_tensor(
                            out=acc[:].rearrange("p (j f) -> p j f", f=HIDDEN),
                            in0=acc[:].rearrange("p (j f) -> p j f", f=HIDDEN),
                            in1=src,
                            op=mybir.AluOpType.add,
                        )

            acc1 = ap_.tile([128, NBLK * HIDDEN], f32, tag="acc1")
            aggregate(tab64_1, pt1, acc1)

            # ---- layer-1 pointwise: q2 = relu(acc*dinv + b1) * dinv --------
            nc.vector.tensor_tensor(
                out=acc1[:], in0=acc1[:], in1=dinvr[:], op=mybir.AluOpType.mult
            )
            nc.vector.tensor_tensor(
                out=acc1[:], in0=acc1[:], in1=b1r[:], op=mybir.AluOpType.add
            )
            nc.vector.tensor_scalar_max(out=acc1[:], in0=acc1[:], scalar1=0.0)
            nc.vector.tensor_tensor(
                out=acc1[:], in0=acc1[:], in1=dinvr[:], op=mybir.AluOpType.mult
            )
            sl = acc1[:, (NBLK - 1) * HIDDEN : NBLK * HIDDEN]
            nc.vector.tensor_scalar_mul(out=sl, in0=sl, scalar1=dmask[:, :1])
            nc.sync.dma_start(
                out=q2d[:].rearrange("(j p) f -> p j f", p=128),
                in_=acc1[:].rearrange("p (j f) -> p j f", f=HIDDEN),
            )

            # ---- allgather 2 + spread --------------------------------------
            nc.gpsimd.collective_compute(
                "AllGather",
                mybir.AluOpType.bypass,
                replica_groups=rg,
                ins=[q2d[:]],
                outs=[tab16_2[:]],
            )
            for w in range(8):
                nc.sync.dma_start(
                    out=tab64_2[w * NPC : (w + 1) * NPC, :HIDDEN],
                    in_=tab16_2[w * NPC : (w + 1) * NPC],
                )

            acc2 = ap_.tile([128, NBLK * HIDDEN], f32, tag="acc2")
            aggregate(tab64_2, pt2, acc2)

            # ---- layer-2 tail: z = (acc*dinv) @ W2 + b2, log_softmax -------
            nc.vector.tensor_tensor(
                out=acc2[:], in0=acc2[:], in1=dinvr[:], op=mybir.AluOpType.mult
            )
            for s in range(n_phA):
                b0 = s * SB
                nblk_s = min(SB, NBLK - b0)
                tT = wp.tile([HIDDEN, SB * 128], f32, tag="tT")
                psO = ppO.tile([128, SB * N_CLASSES], f32, tag="psO")
                for j in range(nblk_s):
                    b = b0 + j
                    psT = ppT.tile([HIDDEN, 128], f32, tag="psT2")
                    nc.tensor.transpose(
                        out=psT[:],
                        in_=acc2[:, b * HIDDEN : (b + 1) * HIDDEN],
                        identity=identf[:],
                    )
                    nc.vector.tensor_copy(
                        out=tT[:, j * 128 : (j + 1) * 128], in_=psT[:]
                    )
                    nc.tensor.matmul(
                        out=psO[:, j * N_CLASSES : (j + 1) * N_CLASSES],
                        lhsT=tT[:, j * 128 : (j + 1) * 128],
                        rhs=W2s[:],
                        start=True,
                        stop=True,
                    )
                z4 = wp.tile([128, SB * N_CLASSES], f32, tag="z4")
                zl = z4[:, : nblk_s * N_CLASSES]
                nc.vector.tensor_tensor(
                    out=zl,
                    in0=psO[:, : nblk_s * N_CLASSES],
                    in1=b2r[:, : nblk_s * N_CLASSES],
                    op=mybir.AluOpType.add,
                )
                negm = wp.tile([128, SB], f32, tag="negm")
                nc.vector.tensor_reduce(
                    out=negm[:, :nblk_s],
                    in_=zl.rearrange("p (n c) -> p n c", c=N_CLASSES),
                    axis=mybir.AxisListType.X,
                    op=mybir.AluOpType.max,
                    negate=True,
                )
                e4 = wp.tile([128, SB * N_CLASSES], f32, tag="e4")
                ssum = wp.tile([128, SB], f32, tag="ssum")
                for j in range(nblk_s):
                    nc.scalar.activation(
                        out=e4[:, j * N_CLASSES : (j + 1) * N_CLASSES],
                        in_=z4[:, j * N_CLASSES : (j + 1) * N_CLASSES],
                        func=mybir.ActivationFunctionType.Exp,
                        bias=negm[:, j : j + 1],
                        scale=1.0,
                        accum_out=ssum[:, j : j + 1],
                    )
                ls = wp.tile([128, SB], f32, tag="ls")
                nc.scalar.activation(
                    out=ls[:, :nblk_s],
                    in_=ssum[:, :nblk_s],
                    func=mybir.ActivationFunctionType.Ln,
                )
                o4 = wp.tile([128, SB * N_CLASSES], f32, tag="o4")
                for j in range(nblk_s):
                    nc.vector.tensor_scalar(
                        out=o4[:, j * N_CLASSES : (j + 1) * N_CLASSES],
                        in0=z4[:, j * N_CLASSES : (j + 1) * N_CLASSES],
                        scalar1=negm[:, j : j + 1],
                        scalar2=ls[:, j : j + 1],
                        op0=mybir.AluOpType.add,
                        op1=mybir.AluOpType.subtract,
                    )
                for j in range(nblk_s):
                    nc.sync.dma_start(
                        out=out_d[b0 + j],
                        in_=o4[:, j * N_CLASSES : (j + 1) * N_CLASSES],
                    )

    nc.finalize()
    return nc


# ----------------------------------------------------------------------------
# entry point
# ----------------------------------------------------------------------------

def kernel(x, edge_index, W1, b1, W2, b2, _trace=False):
    x = np.asarray(x)
    edge_index = np.asarray(edge_index)
    W1 = np.asarray(W1, dtype=np.float32)
    b1 = np.asarray(b1, dtype=np.float32)
    W2 = np.asarray(W2, dtype=np.float32)
    b2 = np.asarray(b2, dtype=np.float32)

    if "meta" not in _cache:
        _cache["meta"] = _preprocess(edge_index)
        _cache["nc"] = _build_program(_cache["meta"])
    meta = _cache["meta"]
    nc = _cache["nc"]

    import ml_dtypes

    W1r = (
        W1.reshape(4, 128, HIDDEN)
        .transpose(1, 0, 2)
        .reshape(128, 64)
        .astype(ml_dtypes.bfloat16)
    )
    b1r = np.tile(b1, (128, NBLK)).astype(np.float32)
    b2r = np.tile(b2, (128, SB)).astype(np.float32)
    W2s = W2.astype(np.float32)
    identf = np.eye(128, dtype=np.float32)
    dmask = np.ones((128, 1), dtype=np.float32)
    dmask[128 - (NPC - NPC_REAL):] = 0.0

    in_maps = []
    for c in range(NCORES):
        lo = c * NPC_REAL
        xc = np.zeros((NPC, N_FEAT), dtype=np.float32)
        xc[:NPC_REAL] = x[lo : lo + NPC_REAL]
        in_maps.append(
            {
                "xT": np.ascontiguousarray(xc.T).astype(ml_dtypes.bfloat16),
                "idx1": meta["idx1"][c],
                "idx2": meta["idx2"][c],
                "degrep": meta["deg_rep"][c],
                "W1r": W1r,
                "b1r": b1r,
                "W2s": W2s,
                "b2r": b2r,
                "identf": identf,
                "dmask": dmask,
            }
        )

    res = run_bass_kernel_spmd(nc, in_maps, list(range(NCORES)), trace=_trace)
    _cache["last_res"] = res

    out = np.empty((N_NODES, N_CLASSES), dtype=np.float32)
    for c in range(NCORES):
        oc = res.results[c]["out"].reshape(NPC, N_CLASSES)
        out[c * NPC_REAL : (c + 1) * NPC_REAL] = oc[:NPC_REAL]
    return out


# revision 16
# speedup vs baseline: 1.3904x; 1.3904x over previous
"""2-layer GCN (100k nodes, 3.2M edges) on 8 Trainium2 NeuronCores.

Strategy (node-partitioned, DMA-gather aggregation):
  - Nodes range-partitioned across 8 cores (12500 real + 44 dummy = 12544
    positions per core, plain order, no global permutation).
  - GCN algebra: out = D^-1/2 A_hat D^-1/2 (H W). Features are pre-scaled by
    dinv before the table, aggregated over in-edges, post-scaled by dinv.
    Layer 2 applies W2 after aggregation (linearity), so both layers
    aggregate 16-wide features.
  - Per layer, each core computes its feature shard node-major, AllGathers it
    ([100352, 16] f32 in DRAM), and spreads it to a 256B-row table
    ([100352, 64] f32) because SWDGE dma_gather descriptors are 256B-aligned.
  - Aggregation is two-level to keep DMA descriptor counts near the edge
    count (int16 gather indices only address 32768 rows, so gathers are
    windowed per source core):
      stage 1: per (window w = src core, group of 4 seg-columns) dma_gather
        pulls each edge's source row; nodes are sorted per-window by their
        window-w in-degree so segment widths stay tight; a strided DVE
        reduce produces per-(node, window) partials written to a local DRAM
        partial table pt[w] in window order.
      stage 2: 8 dma_gathers (one per window, uniform 1 slot/node) pull each
        node's 8 partials back in plain node order; DVE adds combine them.
  - Self-loops are ordinary slots in the own-core window. Padding slots point
    at each core's dummy zero row.

All floating-point math (matmuls, rsqrt, aggregation, bias, relu,
log_softmax) runs on device. The host only restructures integers into the
gather index tensors and relayouts inputs.
"""

import numpy as np

import concourse.bass as bass
import concourse.bacc as bacc
import concourse.mybir as mybir
import concourse.tile as tile
from concourse.bass_utils import run_bass_kernel_spmd

N_NODES = 100000
N_FEAT = 512
HIDDEN = 16
N_CLASSES = 64
NCORES = 8
NPC_REAL = 12500          # real nodes per core
NPC = 12544               # padded positions per core (98 * 128)
NBLK = NPC // 128         # 98 seg-columns / blocks of 128 nodes
NT = NCORES * NPC         # global table rows
EL = 64                   # table row width in f32 (256B rows for dma_gather)
SEG = 4                   # seg-columns per stage-1 gather group
SB = 4                    # blocks per tail-compute group
DUMMY = NPC - 1           # each core's last position is a zero row

_cache = {}


# ----------------------------------------------------------------------------
# host-side graph restructuring (integer work only)
# ----------------------------------------------------------------------------

def _preprocess(edge_index):
    src = edge_index[0].astype(np.int64)
    dst = edge_index[1].astype(np.int64)
    c_dst = dst // NPC_REAL
    l_dst = dst - c_dst * NPC_REAL
    w_src = src // NPC_REAL
    l_src = src - w_src * NPC_REAL

    # per-(core, node, window) message counts, incl. self-loop
    key = (c_dst * NPC + l_dst) * 8 + w_src
    m = np.bincount(key, minlength=NCORES * NPC * 8).reshape(NCORES, NPC, 8)
    for c in range(NCORES):
        m[c, :NPC_REAL, c] += 1

    deg = m.sum(axis=2)                      # in-degree incl self (0 for dummy)

    # per-(core, window) node order by window in-degree desc; dummies last
    m2 = m.transpose(0, 2, 1)                # [c, w, node]
    order = np.argsort(-m2, axis=2, kind="stable")
    rank = np.empty_like(order)
    np.put_along_axis(rank, order, np.broadcast_to(np.arange(NPC), order.shape), axis=2)

    # seg widths (max over 128 sorted nodes), unified across cores
    msort = -np.sort(-m2, axis=2)            # [c, w, node] desc
    segD = msort.reshape(NCORES, 8, NBLK, 128).max(axis=3).max(axis=0)  # [w, seg]

    # stage-1 groups: adaptive, capped so each instruction stays well under
    # the per-queue SWDGE descriptor ring (<= 27 slots/partition = 3456 descs)
    CAP = 27
    segDmax = np.maximum(1, segD.max(axis=0))  # worst window per seg
    groups = []
    b = 0
    while b < NBLK:
        D = segDmax[b]
        nb = 1
        while b + nb < NBLK and (nb + 1) * max(D, segDmax[b + nb]) <= CAP:
            D = max(D, segDmax[b + nb])
            nb += 1
        groups.append((b, nb))
        b += nb
    NG = len(groups)
    Dwg = np.zeros((8, NG), dtype=np.int64)
    for w in range(8):
        for gi, (s0, ns) in enumerate(groups):
            Dwg[w, gi] = max(1, segD[w, s0 : s0 + ns].max())

    # instruction layout: concat (w, g) index streams; ni = 128*ns*D
    ni = np.zeros((8, NG), dtype=np.int64)
    for w in range(8):
        for gi, (s0, ns) in enumerate(groups):
            ni[w, gi] = 128 * ns * Dwg[w, gi]
    base = np.zeros(8 * NG + 1, dtype=np.int64)
    base[1:] = np.cumsum(ni.ravel())
    I1 = int(base[-1])                        # total stage-1 indices per core

    # per-edge slot assignment
    ekey = (c_dst * 8 + w_src) * NPC + l_dst
    perm = np.argsort(ekey, kind="stable")
    ekey_s = ekey[perm]
    lsrc_s = l_src[perm]
    counts = np.bincount(ekey_s, minlength=NCORES * 8 * NPC)
    starts = np.zeros(NCORES * 8 * NPC + 1, dtype=np.int64)
    starts[1:] = np.cumsum(counts)
    j_within = np.arange(len(ekey_s), dtype=np.int64) - starts[ekey_s]

    gi_of_seg = np.zeros(NBLK, dtype=np.int64)
    jloc_of_seg = np.zeros(NBLK, dtype=np.int64)
    for gidx, (s0, ns) in enumerate(groups):
        gi_of_seg[s0 : s0 + ns] = gidx
        jloc_of_seg[s0 : s0 + ns] = np.arange(ns)

    cc = ekey_s // (8 * NPC)
    ww = (ekey_s // NPC) % 8
    ll = ekey_s % NPC
    rk = rank[cc, ww, ll]                     # window-sorted position
    seg = rk // 128
    lane = rk % 128
    gi = gi_of_seg[seg]
    jloc = jloc_of_seg[seg]
    D = Dwg[ww, gi]
    o = (jloc * D + j_within) * 128 + lane    # offset within instruction
    flat = base[ww * NG + gi] + o

    idx1w = np.full((NCORES, 16, I1 // 16), DUMMY, dtype=np.int16)
    idx1w[cc, flat % 16, flat // 16] = lsrc_s.astype(np.int16)

    # self-loop slots
    for c in range(NCORES):
        l = np.arange(NPC_REAL, dtype=np.int64)
        jw = m[c, :NPC_REAL, c] - 1           # last slot in own-core window
        rk = rank[c, c, l]
        seg = rk // 128
        lane = rk % 128
        gi = gi_of_seg[seg]
        jloc = jloc_of_seg[seg]
        D = Dwg[c, gi]
        flat = base[c * NG + gi] + (jloc * D + jw) * 128 + lane
        idx1w[c, flat % 16, flat // 16] = l.astype(np.int16)

    idx1 = np.tile(idx1w, (1, 8, 1))          # replicate over the 8 q7 cores

    # stage-2: per window, plain-order nodes gather their partial's rank
    NI2 = NPC
    idx2w = np.zeros((NCORES, 16, (8 * NI2) // 16), dtype=np.int16)
    for c in range(NCORES):
        for w in range(8):
            n = np.arange(NPC, dtype=np.int64)
            flat = w * NI2 + (n // 128) * 128 * 0 + (n % 128) + (n // 128) * 128
            # flat within window w stream: slotpos = block j = n//128, lane = n%128
            val = rank[c, w, n].astype(np.int16)
            idx2w[c, flat % 16, flat // 16] = val
    idx2 = np.tile(idx2w, (1, 8, 1))

    # degrees, node-major [128 part, NBLK] -> repeated 16x along free
    deg_pb = np.maximum(deg, 1).reshape(NCORES, NBLK, 128).transpose(0, 2, 1)
    deg_rep = np.repeat(deg_pb, HIDDEN, axis=2).astype(np.int32)

    return {
        "idx1": idx1,
        "idx2": idx2,
        "deg_rep": deg_rep,
        "groups": groups,
        "Dwg": Dwg,
        "ni": ni,
        "base": base,
        "I1": I1,
    }


# ----------------------------------------------------------------------------
# device program
# ----------------------------------------------------------------------------

def _build_program(meta):
    groups = meta["groups"]
    Dwg = meta["Dwg"]
    ni = meta["ni"]
    base = meta["base"]
    I1 = meta["I1"]
    NG = len(groups)
    f32 = mybir.dt.float32
    bf16 = mybir.dt.bfloat16
    MAXSLOT = int(ni.max() // 128)
    MAXNS = max(ns for (_, ns) in groups)
    n_phA = (NBLK + SB - 1) // SB

    nc = bacc.Bacc(
        "TRN2", target_bir_lowering=False, debug=False, num_devices=NCORES,
        num_swdge_queues=4,
    )
    xT = nc.declare_dram_parameter("xT", [N_FEAT, NPC], bf16, isOutput=False)
    idx1_in = nc.declare_dram_parameter(
        "idx1", [128, I1 // 16], mybir.dt.int16, isOutput=False
    )
    idx2_in = nc.declare_dram_parameter(
        "idx2", [128, (8 * NPC) // 16], mybir.dt.int16, isOutput=False
    )
    degrep_in = nc.declare_dram_parameter(
        "degrep", [128, NBLK * HIDDEN], mybir.dt.int32, isOutput=False
    )
    W1r_in = nc.declare_dram_parameter("W1r", [128, 64], bf16, isOutput=False)
    b1r_in = nc.declare_dram_parameter("b1r", [128, NBLK * HIDDEN], f32, isOutput=False)
    W2s_in = nc.declare_dram_parameter("W2s", [HIDDEN, N_CLASSES], f32, isOutput=False)
    b2r_in = nc.declare_dram_parameter("b2r", [128, SB * N_CLASSES], f32, isOutput=False)
    identf_in = nc.declare_dram_parameter("identf", [128, 128], f32, isOutput=False)
    dmask_in = nc.declare_dram_parameter("dmask", [128, 1], f32, isOutput=False)
    out_d = nc.declare_dram_parameter("out", [NBLK, 128, N_CLASSES], f32, isOutput=True)

    q1d = nc.dram_tensor("q1d", [NPC, HIDDEN], f32)
    q2d = nc.dram_tensor("q2d", [NPC, HIDDEN], f32)
    tab16_1 = nc.dram_tensor("tab16_1", [NT, HIDDEN], f32, addr_space="Shared")
    tab16_2 = nc.dram_tensor("tab16_2", [NT, HIDDEN], f32, addr_space="Shared")
    tab64_1 = nc.dram_tensor("tab64_1", [NT, EL], f32)
    tab64_2 = nc.dram_tensor("tab64_2", [NT, EL], f32)
    pt1 = nc.dram_tensor("pt1", [8, NPC, EL], f32)
    pt2 = nc.dram_tensor("pt2", [8, NPC, EL], f32)

    rg = [list(range(NCORES))]

    with tile.TileContext(nc) as tc:
        with (
            tc.tile_pool(name="const", bufs=1) as cp,
            tc.tile_pool(name="xt", bufs=2) as xp,
            tc.tile_pool(name="i1", bufs=3) as ip,
            tc.tile_pool(name="msg", bufs=2) as mp,
            tc.tile_pool(name="m2", bufs=2) as m2p,
            tc.tile_pool(name="work", bufs=3) as wp,
            tc.tile_pool(name="acc", bufs=1) as ap_,
            tc.tile_pool(name="ps", bufs=2, space="PSUM") as pp,
            tc.tile_pool(name="psT", bufs=2, space="PSUM") as ppT,
            tc.tile_pool(name="psO", bufs=2, space="PSUM") as ppO,
        ):
            # ---- constants -------------------------------------------------
            W1r = cp.tile([128, 64], bf16)
            nc.sync.dma_start(out=W1r[:], in_=W1r_in[:])
            b1r = cp.tile([128, NBLK * HIDDEN], f32)
            nc.sync.dma_start(out=b1r[:], in_=b1r_in[:])
            W2s = cp.tile([HIDDEN, N_CLASSES], f32)
            nc.sync.dma_start(out=W2s[:], in_=W2s_in[:])
            b2r = cp.tile([128, SB * N_CLASSES], f32)
            nc.sync.dma_start(out=b2r[:], in_=b2r_in[:])
            identf = cp.tile([128, 128], f32)
            nc.sync.dma_start(out=identf[:], in_=identf_in[:])
            dmask = cp.tile([128, 1], f32)
            nc.sync.dma_start(out=dmask[:], in_=dmask_in[:])
            idx2_sb = cp.tile([128, (8 * NPC) // 16], mybir.dt.int16)
            nc.sync.dma_start(out=idx2_sb[:], in_=idx2_in[:])

            degrep = cp.tile([128, NBLK * HIDDEN], mybir.dt.int32)
            nc.sync.dma_start(out=degrep[:], in_=degrep_in[:])
            dinvr = cp.tile([128, NBLK * HIDDEN], f32)
            nc.vector.tensor_copy(out=dinvr[:], in_=degrep[:])
            nc.vector.reciprocal(out=dinvr[:], in_=dinvr[:])
            nc.scalar.activation(
                out=dinvr[:], in_=dinvr[:], func=mybir.ActivationFunctionType.Sqrt
            )

            # ---- phase A: q1 = (x @ W1) * dinv, node-major -----------------
            for s in range(n_phA):
                b0 = s * SB
                nblk_s = min(SB, NBLK - b0)
                w = nblk_s * 128
                xts = []
                for kc in range(4):
                    xt = xp.tile([128, SB * 128], bf16, tag=f"xt{kc}")
                    nc.sync.dma_start(
                        out=xt[:, :w],
                        in_=xT[kc * 128 : (kc + 1) * 128, b0 * 128 : b0 * 128 + w],
                    )
                    xts.append(xt)
                psF = pp.tile([HIDDEN, SB * 128], f32, tag="psF")
                for kc in range(4):
                    nc.tensor.matmul(
                        out=psF[:, :w],
                        lhsT=W1r[:, kc * HIDDEN : (kc + 1) * HIDDEN],
                        rhs=xts[kc][:, :w],
                        start=(kc == 0),
                        stop=(kc == 3),
                    )
                qf = wp.tile([HIDDEN, SB * 128], f32, tag="qf")
                nc.vector.tensor_copy(out=qf[:, :w], in_=psF[:, :w])
                q_nm = wp.tile([128, SB * HIDDEN], f32, tag="qnm")
                for j in range(nblk_s):
                    b = b0 + j
                    psT = ppT.tile([128, HIDDEN], f32, tag="psT")
                    nc.tensor.transpose(
                        out=psT[:],
                        in_=qf[:, j * 128 : (j + 1) * 128],
                        identity=identf[:HIDDEN, :HIDDEN],
                    )
                    nc.vector.tensor_scalar_mul(
                        out=q_nm[:, j * HIDDEN : (j + 1) * HIDDEN],
                        in0=psT[:],
                        scalar1=dinvr[:, b * HIDDEN : b * HIDDEN + 1],
                    )
                nc.sync.dma_start(
                    out=q1d[b0 * 128 : b0 * 128 + w].rearrange(
                        "(j p) f -> p j f", p=128
                    ),
                    in_=q_nm[:, : nblk_s * HIDDEN].rearrange(
                        "p (j f) -> p j f", f=HIDDEN
                    ),
                )

            # ---- allgather + spread to 256B rows ---------------------------
            nc.gpsimd.collective_compute(
                "AllGather",
                mybir.AluOpType.bypass,
                replica_groups=rg,
                ins=[q1d[:]],
                outs=[tab16_1[:]],
            )
            for w in range(8):
                nc.sync.dma_start(
                    out=tab64_1[w * NPC : (w + 1) * NPC, :HIDDEN],
                    in_=tab16_1[w * NPC : (w + 1) * NPC],
                )

            # ---- two-level aggregation -------------------------------------
            def aggregate(tab64, pt, acc):
                # stage 1: per (window, group) gather + segmented reduce
                for w in range(8):
                    for gi, (s0, ns) in enumerate(groups):
                        D = int(Dwg[w, gi])
                        n_idx = int(ni[w, gi])
                        cb = int(base[w * NG + gi])
                        i1 = ip.tile([128, (128 * MAXSLOT) // 16],
                                     mybir.dt.int16, tag="i1")
                        nc.sync.dma_start(
                            out=i1[:, : n_idx // 16],
                            in_=idx1_in[:, cb // 16 : (cb + n_idx) // 16],
                        )
                        msg = mp.tile([128, MAXSLOT * EL], f32, tag="msg")
                        nc.gpsimd.dma_gather(
                            out_ap=msg[:, : ns * D * EL].rearrange(
                                "p (s e) -> p s e", e=EL
                            ),
                            in_ap=tab64[w * NPC : (w + 1) * NPC],
                            idxs_ap=i1[:, : n_idx // 16],
                            num_idxs=n_idx,
                            num_idxs_reg=n_idx,
                            elem_size=EL,
                            single_packet=False,
                            queue_num=(w * NG + gi) % 4,
                        )
                        part = wp.tile([128, MAXNS * HIDDEN], f32, tag="part")
                        nc.vector.tensor_reduce(
                            out=part[:, : ns * HIDDEN].rearrange(
                                "p (j f) -> p j f", f=HIDDEN
                            ),
                            in_=msg[:, : ns * D * EL].rearrange(
                                "p (j s e) -> p j e s", j=ns, e=EL
                            )[:, :, :HIDDEN, :],
                            axis=mybir.AxisListType.X,
                            op=mybir.AluOpType.add,
                        )
                        nc.sync.dma_start(
                            out=pt[w, s0 * 128 : (s0 + ns) * 128, :HIDDEN].rearrange(
                                "(j p) f -> p j f", p=128
                            ),
                            in_=part[:, : ns * HIDDEN].rearrange(
                                "p (j f) -> p j f", f=HIDDEN
                            ),
                        )
                # stage 2: per window, gather partials in plain order, combine
                chunks = [(0, 25), (25, 25), (50, 24), (74, 24)]
                for w in range(8):
                    m2 = m2p.tile([128, NBLK * EL], f32, tag="m2")
                    for ci, (c0, cn) in enumerate(chunks):
                        nc.gpsimd.dma_gather(
                            out_ap=m2[:].rearrange("p (s e) -> p s e", e=EL)[
                                :, c0 : c0 + cn, :
                            ],
                            in_ap=pt[w],
                            idxs_ap=idx2_sb[
                                :,
                                w * (NPC // 16) + c0 * 8 : w * (NPC // 16)
                                + (c0 + cn) * 8,
                            ],
                            num_idxs=cn * 128,
                            num_idxs_reg=cn * 128,
                            elem_size=EL,
                            single_packet=False,
                            queue_num=(w + ci) % 4,
                        )
                    src = m2[:].rearrange("p (j e) -> p j e", e=EL)[:, :, :HIDDEN]
                    if w == 0:
                        nc.vector.tensor_copy(
                            out=acc[:].rearrange("p (j f) -> p j f", f=HIDDEN),
                            in_=src,
                        )
                    else:
                        nc.vector.tensor_tensor(
                            out=acc[:].rearrange("p (j f) -> p j f", f=HIDDEN),
                            in0=acc[:].rearrange("p (j f) -> p j f", f=HIDDEN),
                            in1=src,
                            op=mybir.AluOpType.add,
                        )

            acc1 = ap_.tile([128, NBLK * HIDDEN], f32, tag="acc1")
            aggregate(tab64_1, pt1, acc1)

            # ---- layer-1 pointwise: q2 = relu(acc*dinv + b1) * dinv --------
            nc.vector.tensor_tensor(
                out=acc1[:], in0=acc1[:], in1=dinvr[:], op=mybir.AluOpType.mult
            )
            nc.vector.tensor_tensor(
                out=acc1[:], in0=acc1[:], in1=b1r[:], op=mybir.AluOpType.add
            )
            nc.vector.tensor_scalar_max(out=acc1[:], in0=acc1[:], scalar1=0.0)
            nc.vector.tensor_tensor(
                out=acc1[:], in0=acc1[:], in1=dinvr[:], op=mybir.AluOpType.mult
            )
            sl = acc1[:, (NBLK - 1) * HIDDEN : NBLK * HIDDEN]
            nc.vector.tensor_scalar_mul(out=sl, in0=sl, scalar1=dmask[:, :1])
            nc.sync.dma_start(
                out=q2d[:].rearrange("(j p) f -> p j f", p=128),
                in_=acc1[:].rearrange("p (j f) -> p j f", f=HIDDEN),
            )

            # ---- allgather 2 + spread --------------------------------------
            nc.gpsimd.collective_compute(
                "AllGather",
                mybir.AluOpType.bypass,
                replica_groups=rg,
                ins=[q2d[:]],
                outs=[tab16_2[:]],
            )
            for w in range(8):
                nc.sync.dma_start(
                    out=tab64_2[w * NPC : (w + 1) * NPC, :HIDDEN],
                    in_=tab16_2[w * NPC : (w + 1) * NPC],
                )

            acc2 = ap_.tile([128, NBLK * HIDDEN], f32, tag="acc2")
            aggregate(tab64_2, pt2, acc2)

            # ---- layer-2 tail: z = (acc*dinv) @ W2 + b2, log_softmax -------
            nc.vector.tensor_tensor(
                out=acc2[:], in0=acc2[:], in1=dinvr[:], op=mybir.AluOpType.mult
            )
            for s in range(n_phA):
                b0 = s * SB
                nblk_s = min(SB, NBLK - b0)
                tT = wp.tile([HIDDEN, SB * 128], f32, tag="tT")
                psO = ppO.tile([128, SB * N_CLASSES], f32, tag="psO")
                for j in range(nblk_s):
                    b = b0 + j
                    psT = ppT.tile([HIDDEN, 128], f32, tag="psT2")
                    nc.tensor.transpose(
                        out=psT[:],
                        in_=acc2[:, b * HIDDEN : (b + 1) * HIDDEN],
                        identity=identf[:],
                    )
                    nc.vector.tensor_copy(
                        out=tT[:, j * 128 : (j + 1) * 128], in_=psT[:]
                    )
                    nc.tensor.matmul(
                        out=psO[:, j * N_CLASSES : (j + 1) * N_CLASSES],
                        lhsT=tT[:, j * 128 : (j + 1) * 128],
                        rhs=W2s[:],
                        start=True,
                        stop=True,
                    )
                z4 = wp.tile([128, SB * N_CLASSES], f32, tag="z4")
                zl = z4[:, : nblk_s * N_CLASSES]
                nc.vector.tensor_tensor(
                    out=zl,
                    in0=psO[:, : nblk_s * N_CLASSES],
                    in1=b2r[:, : nblk_s * N_CLASSES],
                    op=mybir.AluOpType.add,
                )
                negm = wp.tile([128, SB], f32, tag="negm")
                nc.vector.tensor_reduce(
                    out=negm[:, :nblk_s],
                    in_=zl.rearrange("p (n c) -> p n c", c=N_CLASSES),
                    axis=mybir.AxisListType.X,
                    op=mybir.AluOpType.max,
                    negate=True,
                )
                e4 = wp.tile([128, SB * N_CLASSES], f32, tag="e4")
                ssum = wp.tile([128, SB], f32, tag="ssum")
                for j in range(nblk_s):
                    nc.scalar.activation(
                        out=e4[:, j * N_CLASSES : (j + 1) * N_CLASSES],
                        in_=z4[:, j * N_CLASSES : (j + 1) * N_CLASSES],
                        func=mybir.ActivationFunctionType.Exp,
                        bias=negm[:, j : j + 1],
                        scale=1.0,
                        accum_out=ssum[:, j : j + 1],
                    )
                ls = wp.tile([128, SB], f32, tag="ls")
                nc.scalar.activation(
                    out=ls[:, :nblk_s],
                    in_=ssum[:, :nblk_s],
                    func=mybir.ActivationFunctionType.Ln,
                )
                o4 = wp.tile([128, SB * N_CLASSES], f32, tag="o4")
                for j in range(nblk_s):
                    nc.vector.tensor_scalar(
                        out=o4[:, j * N_CLASSES : (j + 1) * N_CLASSES],
                        in0=z4[:, j * N_CLASSES : (j + 1) * N_CLASSES],
                        scalar1=negm[:, j : j + 1],
                        scalar2=ls[:, j : j + 1],
                        op0=mybir.AluOpType.add,
                        op1=mybir.AluOpType.subtract,
                    )
                for j in range(nblk_s):
                    nc.sync.dma_start(
                        out=out_d[b0 + j],
                        in_=o4[:, j * N_CLASSES : (j + 1) * N_CLASSES],
                    )

    nc.finalize()
    return nc


# ----------------------------------------------------------------------------
# entry point
# ----------------------------------------------------------------------------

def kernel(x, edge_index, W1, b1, W2, b2, _trace=False):
    x = np.asarray(x)
    edge_index = np.asarray(edge_index)
    W1 = np.asarray(W1, dtype=np.float32)
    b1 = np.asarray(b1, dtype=np.float32)
    W2 = np.asarray(W2, dtype=np.float32)
    b2 = np.asarray(b2, dtype=np.float32)

    if "meta" not in _cache:
        _cache["meta"] = _preprocess(edge_index)
        _cache["nc"] = _build_program(_cache["meta"])
    meta = _cache["meta"]
    nc = _cache["nc"]

    import ml_dtypes

    W1r = (
        W1.reshape(4, 128, HIDDEN)
        .transpose(1, 0, 2)
        .reshape(128, 64)
        .astype(ml_dtypes.bfloat16)
    )
    b1r = np.tile(b1, (128, NBLK)).astype(np.float32)
    b2r = np.tile(b2, (128, SB)).astype(np.float32)
    W2s = W2.astype(np.float32)
    identf = np.eye(128, dtype=np.float32)
    dmask = np.ones((128, 1), dtype=np.float32)
    dmask[128 - (NPC - NPC_REAL):] = 0.0

    in_maps = []
    for c in range(NCORES):
        lo = c * NPC_REAL
        xc = np.zeros((NPC, N_FEAT), dtype=np.float32)
        xc[:NPC_REAL] = x[lo : lo + NPC_REAL]
        in_maps.append(
            {
                "xT": np.ascontiguousarray(xc.T).astype(ml_dtypes.bfloat16),
                "idx1": meta["idx1"][c],
                "idx2": meta["idx2"][c],
                "degrep": meta["deg_rep"][c],
                "W1r": W1r,
                "b1r": b1r,
                "W2s": W2s,
                "b2r": b2r,
                "identf": identf,
                "dmask": dmask,
            }
        )

    res = run_bass_kernel_spmd(nc, in_maps, list(range(NCORES)), trace=_trace)
    _cache["last_res"] = res

    out = np.empty((N_NODES, N_CLASSES), dtype=np.float32)
    for c in range(NCORES):
        oc = res.results[c]["out"].reshape(NPC, N_CLASSES)
        out[c * NPC_REAL : (c + 1) * NPC_REAL] = oc[:NPC_REAL]
    return out


# revision 17
# speedup vs baseline: 2.1359x; 1.5362x over previous
"""2-layer GCN (100k nodes, 3.2M edges) on 8 Trainium2 NeuronCores.

Strategy (graph/data parallel, per the node-partition + halo-exchange hint):
  - Nodes are range-partitioned across the 8 cores (12500 each + 44 dummies
    -> 12544 = 98*128 positions per core).
  - GCN algebra: out = D^-1/2 A_hat D^-1/2 (H W).  We pre-scale each node's
    transformed features by dinv, segment-sum over in-edges, and post-scale
    by dinv; for layer 2 we aggregate first and apply W2 after (linearity),
    so both layers aggregate 16-dim features.
  - Per layer, each core computes its shard of the (scaled) feature table,
    the shards are AllGather'd (the halo exchange: feature-major [16, 12544]
    f32 per core -> [128, 12544] global table resident in SBUF).
  - Aggregation: edges are grouped by the core that owns their SRC (= the
    16-partition GPSIMD group holding that core's table slice).  Each group
    gathers its edges' source features with the ap_gather ucode; per-node
    slot counts are padded to a uniform width per 128-node block (nodes
    degree-sorted so padding is small); a DVE segmented reduce produces
    per-group partial sums; a PE matmul against a replicated selector
    (layer 1) or replicated W2 (layer 2) sums across the 8 groups.

All floating-point arithmetic (matmuls, degree->rsqrt, aggregation, bias,
relu, log_softmax) runs on device.  The host only restructures integers
(edge lists -> per-block index tensors) and permutes/relayouts tensors.
"""

import numpy as np

import concourse.bass as bass
import concourse.bacc as bacc
import concourse.mybir as mybir
import concourse.tile as tile
from concourse.bass_utils import run_bass_kernel_spmd

N_NODES = 100000
N_FEAT = 512
HIDDEN = 16
N_CLASSES = 64
NCORES = 8
NPC_REAL = 12500          # real nodes per core
NPC = 12544               # padded positions per core (98 * 128)
NBLK = NPC // 128         # 98 blocks of 128 nodes
SB = 4                    # blocks per super-block (ap_gather/reduce batch)
DUMMY_COL = NPC - 1       # every core's last position is a dummy (zero) node

_cache = {}


# ----------------------------------------------------------------------------
# host-side graph restructuring (integer work only)
# ----------------------------------------------------------------------------

def _preprocess(edge_index):
    src = edge_index[0].astype(np.int64)
    dst = edge_index[1].astype(np.int64)

    # in-degree INCLUDES the self-loop; but self-loop edges are handled
    # locally (shard add), not gathered, so they are excluded from the slots
    deg = np.bincount(dst, minlength=N_NODES) + 1

    owner_src = src // NPC_REAL

    m = np.bincount(dst * 8 + owner_src, minlength=N_NODES * 8).reshape(
        N_NODES, 8
    )                                                            # per-group counts
    dtil = m.max(axis=1)                                         # slots per node

    # per-core permutation: sort local nodes by dtil desc; dummies (dtil=-1) last
    order = np.empty((NCORES, NPC), dtype=np.int64)   # position -> local node id
    rank = np.empty(N_NODES, dtype=np.int64)          # global node -> position
    for c in range(NCORES):
        lo = c * NPC_REAL
        d_loc = np.concatenate(
            [dtil[lo : lo + NPC_REAL], np.full(NPC - NPC_REAL, -1, np.int64)]
        )
        o = np.argsort(-d_loc, kind="stable")
        order[c] = o
        inv = np.empty(NPC, dtype=np.int64)
        inv[o] = np.arange(NPC)
        rank[lo : lo + NPC_REAL] = inv[:NPC_REAL]

    # block widths, unified across cores; grouped into super-blocks
    # dtil at position (c, pos): for a block the max is at its first position
    dtil_pos = np.zeros((NCORES, NPC), dtype=np.int64)
    for c in range(NCORES):
        lo = c * NPC_REAL
        real = order[c] < NPC_REAL
        dtil_pos[c][real] = dtil[lo + order[c][real]]
    # adaptive super-blocks: pack consecutive blocks while nodes*width <= cap
    # (amortizes the ~1us fixed cost per ap_gather instruction)
    blk_D = np.zeros(NBLK, dtype=np.int64)
    for b in range(NBLK):
        blk_D[b] = max(1, dtil_pos[:, b * 128 : (b + 1) * 128].max())
    NI_CAP = max(4096, int(128 * blk_D.max()))
    supers = []  # (b0, nblk, D)
    b = 0
    while b < NBLK:
        D = blk_D[b]
        nb = 1
        while (
            b + nb < NBLK
            and nb < SB
            and (nb + 1) * 128 * max(D, blk_D[b + nb]) <= NI_CAP
        ):
            D = max(D, blk_D[b + nb])
            nb += 1
        supers.append((b, nb, int(D)))
        b += nb
    n_super = len(supers)
    sup_of_blk = np.zeros(NBLK, dtype=np.int64)
    for si, (b0, nb, D) in enumerate(supers):
        sup_of_blk[b0 : b0 + nb] = si
    DSUP = np.array([D for (_, _, D) in supers], dtype=np.int64)
    sup_b0 = np.array([b0 for (b0, _, _) in supers], dtype=np.int64)
    num_idxs = np.array([nb * 128 * D for (_, nb, D) in supers], dtype=np.int64)
    colbase = np.zeros(n_super + 1, dtype=np.int64)
    colbase[1:] = np.cumsum(num_idxs // 16)
    IDXCOLS = int(colbase[-1])

    # per-edge slot assignment (vectorized)
    key = dst * 8 + owner_src
    perm = np.argsort(key, kind="stable")
    key_s = key[perm]
    src_s = src[perm]
    starts = np.zeros(N_NODES * 8 + 1, dtype=np.int64)
    starts[1:] = np.cumsum(m.ravel())
    j_within = np.arange(len(src_s), dtype=np.int64) - starts[key_s]

    dst_s = key_s // 8
    g_s = key_s % 8
    c_s = dst_s // NPC_REAL
    pos_s = rank[dst_s]                               # position within core
    blk_s = pos_s // 128
    i_s = pos_s % 128
    sup_s = sup_of_blk[blk_s]
    node_in_sup = (blk_s - sup_b0[sup_s]) * 128 + i_s
    e_col = node_in_sup * DSUP[sup_s] + j_within      # column within instruction
    part = 16 * g_s + (e_col % 16)
    col = colbase[sup_s] + e_col // 16
    val = rank[src_s]                                 # table column of the source

    idx_all = np.full((NCORES, 128, IDXCOLS), DUMMY_COL, dtype=np.int16)
    idx_all[c_s, part, col] = val.astype(np.int16)

    # per-core degree tensors in (partition, block) layout
    deg_pb = np.zeros((NCORES, 128, NBLK), dtype=np.int32)
    for c in range(NCORES):
        lo = c * NPC_REAL
        real = order[c] < NPC_REAL
        d = np.zeros(NPC, dtype=np.int32)
        d[real] = deg[lo + order[c][real]].astype(np.int32)
        deg_pb[c] = d.reshape(NBLK, 128).T            # pos = b*128 + p
    # deg repeated 16x along free dim for batched layer-1 scaling
    deg_rep = np.repeat(deg_pb, HIDDEN, axis=2).reshape(NCORES, 128, NBLK * HIDDEN)
    # note: repeat on axis=2 of [C,128,NBLK] gives [C,128,NBLK*16] with each
    # block's degree contiguous 16 wide -- matches q layout [128, (b f)]

    return {
        "order": order,
        "idx_all": idx_all,
        "deg_pb": deg_pb,
        "deg_rep": deg_rep,
        "supers": supers,
        "num_idxs": num_idxs,
        "colbase": colbase,
        "IDXCOLS": IDXCOLS,
        "NI_CAP": NI_CAP,
    }


# ----------------------------------------------------------------------------
# device program
# ----------------------------------------------------------------------------

def _build_program(meta):
    supers = meta["supers"]
    num_idxs = meta["num_idxs"]
    colbase = meta["colbase"]
    IDXCOLS = meta["IDXCOLS"]
    NI_CAP = meta["NI_CAP"]
    n_phA = (NBLK + SB - 1) // SB      # phase-A block groups (fixed SB)
    f32 = mybir.dt.float32

    nc = bacc.Bacc(
        "TRN2", target_bir_lowering=False, debug=False, num_devices=NCORES
    )
    xT = nc.declare_dram_parameter("xT", [N_FEAT, NPC], f32, isOutput=False)
    idx_in = nc.declare_dram_parameter(
        "idx_in", [128, IDXCOLS], mybir.dt.int16, isOutput=False
    )
    degrep_in = nc.declare_dram_parameter(
        "degrep_in", [128, NBLK * HIDDEN], mybir.dt.int32, isOutput=False
    )
    W1r_in = nc.declare_dram_parameter("W1r", [128, 64], f32, isOutput=False)
    b1r_in = nc.declare_dram_parameter("b1r", [128, SB * HIDDEN], f32, isOutput=False)
    E8I_in = nc.declare_dram_parameter("E8I", [128, HIDDEN], f32, isOutput=False)
    W2r_in = nc.declare_dram_parameter("W2r", [128, N_CLASSES], f32, isOutput=False)
    b2r_in = nc.declare_dram_parameter(
        "b2r", [128, SB * N_CLASSES], f32, isOutput=False
    )
    ident_in = nc.declare_dram_parameter("ident", [128, 128], f32, isOutput=False)
    dmask_in = nc.declare_dram_parameter("dmask", [128, 1], f32, isOutput=False)
    out_d = nc.declare_dram_parameter("out", [NBLK, 128, N_CLASSES], f32, isOutput=True)

    q1d = nc.dram_tensor("q1d", [16, NPC], f32)
    q2d = nc.dram_tensor("q2d", [16, NPC], f32)
    tab1d = nc.dram_tensor("tab1d", [128, NPC], f32, addr_space="Shared")
    tab2d = nc.dram_tensor("tab2d", [128, NPC], f32, addr_space="Shared")

    rg = [list(range(NCORES))]

    with tile.TileContext(nc) as tc:
        with (
            tc.tile_pool(name="const", bufs=1) as cp,
            tc.tile_pool(name="xt", bufs=2) as xp,
            tc.tile_pool(name="msg", bufs=2) as mp,
            tc.tile_pool(name="work", bufs=3) as wp,
            tc.tile_pool(name="shard", bufs=1) as sp,
            tc.tile_pool(name="tab", bufs=1) as tp,
            tc.tile_pool(name="ps", bufs=2, space="PSUM") as pp,
            tc.tile_pool(name="psT", bufs=2, space="PSUM") as ppT,
            tc.tile_pool(name="psO", bufs=2, space="PSUM") as ppO,
        ):
            # ---- constants -------------------------------------------------
            W1r = cp.tile([128, 64], f32)
            nc.sync.dma_start(out=W1r[:], in_=W1r_in[:])
            b1r = cp.tile([128, SB * HIDDEN], f32)
            nc.sync.dma_start(out=b1r[:], in_=b1r_in[:])
            E8I = cp.tile([128, HIDDEN], f32)
            nc.sync.dma_start(out=E8I[:], in_=E8I_in[:])
            W2r = cp.tile([128, N_CLASSES], f32)
            nc.sync.dma_start(out=W2r[:], in_=W2r_in[:])
            b2r = cp.tile([128, SB * N_CLASSES], f32)
            nc.sync.dma_start(out=b2r[:], in_=b2r_in[:])
            ident = cp.tile([128, 128], f32)
            nc.sync.dma_start(out=ident[:], in_=ident_in[:])
            dmask = cp.tile([128, 1], f32)
            nc.sync.dma_start(out=dmask[:], in_=dmask_in[:])
            idx_sb = cp.tile([128, IDXCOLS], mybir.dt.int16)
            nc.sync.dma_start(out=idx_sb[:], in_=idx_in[:])

            # dinv (repeated 16x per block): rsqrt(max(deg,1)) on device
            degrep = cp.tile([128, NBLK * HIDDEN], mybir.dt.int32)
            nc.sync.dma_start(out=degrep[:], in_=degrep_in[:])
            dinvr = cp.tile([128, NBLK * HIDDEN], f32)
            nc.vector.tensor_copy(out=dinvr[:], in_=degrep[:])
            nc.vector.tensor_scalar_max(out=dinvr[:], in0=dinvr[:], scalar1=1.0)
            nc.vector.reciprocal(out=dinvr[:], in_=dinvr[:])
            nc.scalar.activation(
                out=dinvr[:], in_=dinvr[:], func=mybir.ActivationFunctionType.Sqrt
            )

            shard = sp.tile([16, NPC], f32)   # feat-major shard (reused q1/q2)
            table = tp.tile([128, NPC], f32)  # gathered global table

            def post_to_shard(qa4, b0, nblk_s):
                """transpose node-major [128, nblk_s*16] -> shard strips."""
                for j in range(nblk_s):
                    b = b0 + j
                    psT = ppT.tile([HIDDEN, 128], f32, tag="psT")
                    nc.tensor.transpose(
                        out=psT[:],
                        in_=qa4[:, j * HIDDEN : (j + 1) * HIDDEN],
                        identity=ident[:],
                    )
                    nc.vector.tensor_copy(
                        out=shard[:, b * 128 : (b + 1) * 128], in_=psT[:]
                    )

            # ---- phase A: q1 = (x @ W1) * dinv, feat-major shard -----------
            for s in range(n_phA):
                b0 = s * SB
                nblk_s = min(SB, NBLK - b0)
                w = nblk_s * 128
                xts = []
                for kc in range(4):
                    xt = xp.tile([128, SB * 128], f32, tag=f"xt{kc}")
                    nc.sync.dma_start(
                        out=xt[:, :w],
                        in_=xT[kc * 128 : (kc + 1) * 128, b0 * 128 : b0 * 128 + w],
                    )
                    xts.append(xt)
                qa4 = wp.tile([128, SB * HIDDEN], f32, tag="qa4")
                for j in range(nblk_s):
                    b = b0 + j
                    psA = pp.tile([128, HIDDEN], f32, tag="psA")
                    for kc in range(4):
                        nc.tensor.matmul(
                            out=psA[:],
                            lhsT=xts[kc][:, j * 128 : (j + 1) * 128],
                            rhs=W1r[:, kc * HIDDEN : (kc + 1) * HIDDEN],
                            start=(kc == 0),
                            stop=(kc == 3),
                        )
                    nc.vector.tensor_tensor(
                        out=qa4[:, j * HIDDEN : (j + 1) * HIDDEN],
                        in0=psA[:],
                        in1=dinvr[:, b * HIDDEN : (b + 1) * HIDDEN],
                        op=mybir.AluOpType.mult,
                    )
                post_to_shard(qa4, b0, nblk_s)
            nc.sync.dma_start(out=q1d[:], in_=shard[:])

            # ---- allgather 1 + table load ---------------------------------
            nc.gpsimd.collective_compute(
                "AllGather",
                mybir.AluOpType.bypass,
                replica_groups=rg,
                ins=[q1d[:]],
                outs=[tab1d[:]],
            )
            nc.sync.dma_start(out=table[:], in_=tab1d[:])

            # ---- aggregation helper ---------------------------------------
            def aggregate(s):
                """gather + segmented reduce; returns [128, nodes] partials."""
                b0, nblk_s, D = supers[s]
                nodes = nblk_s * 128
                ni = int(num_idxs[s])
                msg = mp.tile([128, NI_CAP], f32, tag="msg")
                nc.gpsimd.ap_gather(
                    out_ap=msg[:, :ni],
                    in_ap=table[:],
                    idxs_ap=idx_sb[:, int(colbase[s]) : int(colbase[s + 1])],
                    channels=128,
                    num_elems=NPC,
                    d=1,
                    num_idxs=ni,
                )
                part = wp.tile([128, SB * 128], f32, tag="part")
                nc.vector.tensor_reduce(
                    out=part[:, :nodes],
                    in_=msg[:, :ni].rearrange("p (n d) -> p n d", d=D),
                    axis=mybir.AxisListType.X,
                    op=mybir.AluOpType.add,
                )
                # self-loop contribution: q[n] is resident in the local shard;
                # add it into one group's partial rows (the cross-group matmul
                # sums over all 8 groups, so any one group works)
                nc.vector.tensor_tensor(
                    out=part[0:16, :nodes],
                    in0=part[0:16, :nodes],
                    in1=shard[:, b0 * 128 : b0 * 128 + nodes],
                    op=mybir.AluOpType.add,
                )
                return part, b0, nblk_s

            # ---- layer 1 aggregation -> q2 shard --------------------------
            for s in range(len(supers)):
                part, b0, nblk_s = aggregate(s)
                psX = pp.tile([128, SB * HIDDEN], f32, tag="psA")
                for j in range(nblk_s):
                    nc.tensor.matmul(
                        out=psX[:, j * HIDDEN : (j + 1) * HIDDEN],
                        lhsT=part[:, j * 128 : (j + 1) * 128],
                        rhs=E8I[:],
                        start=True,
                        stop=True,
                    )
                qa4 = wp.tile([128, SB * HIDDEN], f32, tag="qa4")
                dslice = dinvr[:, b0 * HIDDEN : b0 * HIDDEN + nblk_s * HIDDEN]
                ql = qa4[:, : nblk_s * HIDDEN]
                nc.vector.tensor_tensor(
                    out=ql, in0=psX[:, : nblk_s * HIDDEN], in1=dslice,
                    op=mybir.AluOpType.mult,
                )
                nc.vector.tensor_tensor(
                    out=ql, in0=ql, in1=b1r[:, : nblk_s * HIDDEN],
                    op=mybir.AluOpType.add,
                )
                nc.vector.tensor_scalar_max(out=ql, in0=ql, scalar1=0.0)
                nc.vector.tensor_tensor(
                    out=ql, in0=ql, in1=dslice, op=mybir.AluOpType.mult
                )
                if b0 + nblk_s == NBLK:  # kill dummy nodes (last block tail)
                    sl = qa4[:, (nblk_s - 1) * HIDDEN : nblk_s * HIDDEN]
                    nc.vector.tensor_scalar_mul(out=sl, in0=sl, scalar1=dmask[:, :1])
                post_to_shard(qa4, b0, nblk_s)
            nc.sync.dma_start(out=q2d[:], in_=shard[:])

            # ---- allgather 2 + table load ---------------------------------
            nc.gpsimd.collective_compute(
                "AllGather",
                mybir.AluOpType.bypass,
                replica_groups=rg,
                ins=[q2d[:]],
                outs=[tab2d[:]],
            )
            nc.sync.dma_start(out=table[:], in_=tab2d[:])

            # ---- layer 2 aggregation -> logits -> log_softmax -------------
            for s in range(len(supers)):
                part, b0, nblk_s = aggregate(s)
                psO = ppO.tile([128, SB * N_CLASSES], f32, tag="psO")
                for j in range(nblk_s):
                    nc.tensor.matmul(
                        out=psO[:, j * N_CLASSES : (j + 1) * N_CLASSES],
                        lhsT=part[:, j * 128 : (j + 1) * 128],
                        rhs=W2r[:],
                        start=True,
                        stop=True,
                    )
                z4 = wp.tile([128, SB * N_CLASSES], f32, tag="z4")
                for j in range(nblk_s):
                    b = b0 + j
                    nc.vector.tensor_scalar_mul(
                        out=z4[:, j * N_CLASSES : (j + 1) * N_CLASSES],
                        in0=psO[:, j * N_CLASSES : (j + 1) * N_CLASSES],
                        scalar1=dinvr[:, b * HIDDEN : b * HIDDEN + 1],
                    )
                zl = z4[:, : nblk_s * N_CLASSES]
                nc.vector.tensor_tensor(
                    out=zl, in0=zl, in1=b2r[:, : nblk_s * N_CLASSES],
                    op=mybir.AluOpType.add,
                )
                negm = wp.tile([128, SB], f32, tag="negm")
                nc.vector.tensor_reduce(
                    out=negm[:, :nblk_s],
                    in_=zl.rearrange("p (n c) -> p n c", c=N_CLASSES),
                    axis=mybir.AxisListType.X,
                    op=mybir.AluOpType.max,
                    negate=True,
                )
                e4 = wp.tile([128, SB * N_CLASSES], f32, tag="e4")
                ssum = wp.tile([128, SB], f32, tag="ssum")
                for j in range(nblk_s):
                    nc.scalar.activation(
                        out=e4[:, j * N_CLASSES : (j + 1) * N_CLASSES],
                        in_=z4[:, j * N_CLASSES : (j + 1) * N_CLASSES],
                        func=mybir.ActivationFunctionType.Exp,
                        bias=negm[:, j : j + 1],
                        scale=1.0,
                        accum_out=ssum[:, j : j + 1],
                    )
                ls = wp.tile([128, SB], f32, tag="ls")
                nc.scalar.activation(
                    out=ls[:, :nblk_s],
                    in_=ssum[:, :nblk_s],
                    func=mybir.ActivationFunctionType.Ln,
                )
                o4 = wp.tile([128, SB * N_CLASSES], f32, tag="o4")
                for j in range(nblk_s):
                    nc.vector.tensor_scalar(
                        out=o4[:, j * N_CLASSES : (j + 1) * N_CLASSES],
                        in0=z4[:, j * N_CLASSES : (j + 1) * N_CLASSES],
                        scalar1=negm[:, j : j + 1],
                        scalar2=ls[:, j : j + 1],
                        op0=mybir.AluOpType.add,
                        op1=mybir.AluOpType.subtract,
                    )
                for j in range(nblk_s):
                    nc.sync.dma_start(
                        out=out_d[b0 + j],
                        in_=o4[:, j * N_CLASSES : (j + 1) * N_CLASSES],
                    )

    nc.finalize()
    return nc


# ----------------------------------------------------------------------------
# entry point
# ----------------------------------------------------------------------------

def kernel(x, edge_index, W1, b1, W2, b2, _trace=False):
    x = np.asarray(x)
    edge_index = np.asarray(edge_index)
    W1 = np.asarray(W1, dtype=np.float32)
    b1 = np.asarray(b1, dtype=np.float32)
    W2 = np.asarray(W2, dtype=np.float32)
    b2 = np.asarray(b2, dtype=np.float32)

    if "meta" not in _cache:
        _cache["meta"] = _preprocess(edge_index)
        _cache["nc"] = _build_program(_cache["meta"])
    meta = _cache["meta"]
    nc = _cache["nc"]
    order = meta["order"]

    W1r = (
        W1.reshape(4, 128, HIDDEN).transpose(1, 0, 2).reshape(128, 64).astype(
            np.float32
        )
    )
    b1r = np.tile(b1, (128, SB)).astype(np.float32)
    b2r = np.tile(b2, (128, SB)).astype(np.float32)
    f_idx = np.arange(128) % HIDDEN
    E8I = np.eye(HIDDEN, dtype=np.float32)[f_idx]          # [128, 16]
    W2r = W2[f_idx].astype(np.float32)                      # [128, 64]
    ident = np.eye(128, dtype=np.float32)
    dmask = np.ones((128, 1), dtype=np.float32)
    dmask[128 - (NPC - NPC_REAL) :] = 0.0

    in_maps = []
    for c in range(NCORES):
        lo = c * NPC_REAL
        xc = np.zeros((NPC, N_FEAT), dtype=np.float32)
        real = order[c] < NPC_REAL
        xc[real] = x[lo + order[c][real]]
        in_maps.append(
            {
                "xT": np.ascontiguousarray(xc.T),
                "idx_in": meta["idx_all"][c],
                "degrep_in": meta["deg_rep"][c],
                "W1r": W1r,
                "b1r": b1r,
                "E8I": E8I,
                "W2r": W2r,
                "b2r": b2r,
                "ident": ident,
                "dmask": dmask,
            }
        )

    res = run_bass_kernel_spmd(nc, in_maps, list(range(NCORES)), trace=_trace)
    _cache["last_res"] = res

    out = np.empty((N_NODES, N_CLASSES), dtype=np.float32)
    for c in range(NCORES):
        oc = res.results[c]["out"].reshape(NPC, N_CLASSES)  # position-major
        lo = c * NPC_REAL
        real = order[c] < NPC_REAL
        out[lo + order[c][real]] = oc[real]
    return out

